# revision 25
# baseline (speedup 1.0000x reference)
"""Trainium2 Bass kernel for the entity-assignment loss.

Math: per sample b, C[i,j] = mean_d (yt[b,i,d]-yp[b,j,d])^2.
loss = mean_b ( min_perm sum_i C[i, perm(i)] / 8 ).

Since each permutation uses every row i and every column j exactly once,
  sum_i C[i, perm(i)] = (nt + np - 2 * sum_i dot(i, perm(i))) / 64
with nt = sum_i |yt_i|^2, np = sum_j |yp_j|^2 (per-sample constants).
So min over perms only needs MAX over perms of the dot sum, computed with a
2^8 bitmask DP whose bit-i update is a perfectly strided access pattern.

Sharding: pure data parallelism, 256 samples per core across 8 cores; the
final mean is taken on the host from per-sample partial results.
"""

import os
import sys

if "/opt/trn_rl_repo" not in sys.path:
    sys.path.insert(0, "/opt/trn_rl_repo")

import numpy as np

F16 = os.environ.get("K_F16", "1") == "1"    # fp16 streams for DVE 2x modes
DPTT = os.environ.get("K_DPTT", "0") == "1"  # chunk-merged TT-based DP (slower)

B, N, D = 2048, 8, 64
N_CORES = 8
B_LOC = B // N_CORES        # 256 samples per core
N_TILES = B_LOC // 128      # 2 partition tiles of 128 samples
NEG = -1.0e30

TRACE = False
_CACHE = {}


def _build():
    import concourse.bacc as bacc
    import concourse.mybir as mybir
    from concourse.tile import TileContext

    f32 = mybir.dt.float32
    f16 = mybir.dt.float16 if F16 else mybir.dt.float32
    neg = -60000.0 if F16 else NEG
    Alu = mybir.AluOpType
    Act = mybir.ActivationFunctionType
    NT = N_TILES

    nc = bacc.Bacc("TRN2", target_bir_lowering=False, debug=False)
    yt_d = nc.declare_dram_parameter("yt", [B_LOC, N * D], f32, isOutput=False)
    yp_d = nc.declare_dram_parameter("yp", [B_LOC, N * D], f32, isOutput=False)
    out_d = nc.declare_dram_parameter("out", [128, NT], f32, isOutput=True)

    with TileContext(nc) as tc:
        with (
            tc.tile_pool(name="io", bufs=2) as io_pool,
            tc.tile_pool(name="work", bufs=2) as work_pool,
            tc.tile_pool(name="res", bufs=1) as res_pool,
        ):
            loss_t = res_pool.tile([128, NT], f32, tag="loss")
            s_all = res_pool.tile([128, NT], f32, tag="s_all")
            G32 = res_pool.tile([128, NT * N * N], f32, tag="G32")
            eighth_all = res_pool.tile([128, NT * N * N * D // 8], f16, tag="e8")
            for c in range(NT):
                yt_t = io_pool.tile([128, N * D], f32, tag="yt")
                yp_t = io_pool.tile([128, N * D], f32, tag="yp")
                nc.sync.dma_start(out=yt_t[:, :], in_=yt_d[c * 128:(c + 1) * 128, :])
                nc.sync.dma_start(out=yp_t[:, :], in_=yp_d[c * 128:(c + 1) * 128, :])

                # casts first: the DVE multiply waits on these, while the
                # norm totals are only needed at the very end
                yt_f32, yp_f32 = yt_t, yp_t
                if F16:
                    yth = work_pool.tile([128, N * D], f16, tag="yth")
                    yph = work_pool.tile([128, N * D], f16, tag="yph")
                    cast_eng = nc.vector if c == 0 else nc.scalar
                    if c == 0:
                        nc.vector.tensor_copy(yth[:, :], yt_f32[:, :])
                        nc.vector.tensor_copy(yph[:, :], yp_f32[:, :])
                    else:
                        nc.scalar.activation(out=yth[:, :], in_=yt_f32[:, :],
                                             func=Act.Identity)
                        nc.scalar.activation(out=yph[:, :], in_=yp_f32[:, :],
                                             func=Act.Identity)
                    yt_t, yp_t = yth, yph

                # per-sample squared-norm totals on ScalarE (overlapped)
                sq = work_pool.tile([128, N * D], f32, tag="sq")
                nt = work_pool.tile([128, 1], f32, tag="nt")
                npt = work_pool.tile([128, 1], f32, tag="npt")
                nc.scalar.activation(out=sq[:, :], in_=yt_f32[:, :], func=Act.Square,
                                     accum_out=nt[:, 0:1])
                nc.scalar.activation(out=sq[:, :], in_=yp_f32[:, :], func=Act.Square,
                                     accum_out=npt[:, 0:1])
                nc.gpsimd.tensor_add(s_all[:, c:c + 1], nt[:, 0:1], npt[:, 0:1])

                # dots: broadcast multiply, two binary folds over d, then
                # a 16-wide segmented reduce
                yt_b = yt_t.rearrange("p (i d) -> p i d", d=D).unsqueeze(2) \
                    .broadcast_to([128, N, N, D])
                yp_b = yp_t.rearrange("p (j d) -> p j d", d=D).unsqueeze(1) \
                    .broadcast_to([128, N, N, D])
                prod = work_pool.tile([128, N * N * D], f16, tag="prod")
                nc.vector.tensor_tensor(
                    out=prod.rearrange("p (i j d) -> p i j d", j=N, d=D),
                    in0=yt_b, in1=yp_b, op=Alu.mult)
                pv = prod.rearrange("p (q d) -> p q d", d=D)
                half = work_pool.tile([128, N * N * D // 2], f16, tag="half")
                hv = half.rearrange("p (q d) -> p q d", d=D // 2)
                nc.vector.tensor_tensor(
                    out=hv, in0=pv[:, :, 0:D // 2], in1=pv[:, :, D // 2:D],
                    op=Alu.add)
                quart = work_pool.tile([128, N * N * D // 4], f16, tag="quart")
                qv = quart.rearrange("p (q d) -> p q d", d=D // 4)
                nc.vector.tensor_tensor(
                    out=qv, in0=hv[:, :, 0:D // 4], in1=hv[:, :, D // 4:D // 2],
                    op=Alu.add)
                half_sz = N * N * D // 8
                ev = eighth_all[:, c * half_sz:(c + 1) * half_sz] \
                    .rearrange("p (q d) -> p q d", d=D // 8)
                nc.vector.tensor_tensor(
                    out=ev, in0=qv[:, :, 0:D // 8], in1=qv[:, :, D // 4 - D // 8:D // 4],
                    op=Alu.add)
            nc.vector.tensor_reduce(
                out=G32[:, :],
                in_=eighth_all.rearrange("p (q d) -> p q d", d=D // 8),
                axis=mybir.AxisListType.X, op=Alu.add)

            # DP over both chunks jointly: states laid out [chunk, state]
            if DPTT:
                G16 = res_pool.tile([128, NT * N * N], f16, tag="G16")
                nc.vector.tensor_copy(G16[:, :], G32[:, :])
                g_v = G16.rearrange("p (h q) -> p h q", h=NT)
            else:
                g_v = G32.rearrange("p (h q) -> p h q", h=NT)

            dpa = res_pool.tile([128, NT * 256], f16, tag="dpa")
            dpb = res_pool.tile([128, NT * 256], f16, tag="dpb")
            nc.gpsimd.memset(dpa[:, :], neg)
            nc.gpsimd.memset(dpb[:, :], neg)
            cand = res_pool.tile([128, NT * 128], f16, tag="cand")
            bufs = [dpa, dpb]
            for k in range(N):
                old = bufs[k % 2]
                new = bufs[(k + 1) % 2]
                for i in range(N):
                    ci = 2 ** i
                    col = i * N + k
                    gb1 = g_v[:, :, col:col + 1]
                    if k == 0:
                        # sole candidate for each singleton: overwrite, no max.
                        # dp0[empty]=0 is implicit: cand = 0 + G[i,0].
                        tgt = new.rearrange("p (h s) -> p h s", h=NT)[:, :, ci:ci + 1]
                        nc.vector.tensor_copy(tgt, gb1)
                        continue
                    elif k == N - 1:
                        # final column: collect the 8 candidates densely; the
                        # max and the loss combine happen after the loop.
                        ov = old.rearrange("p (h s) -> p h s", h=NT)
                        src = ov[:, :, 255 - ci:256 - ci]
                        cv = cand.rearrange("p (h s) -> p h s", h=NT)[:, :, i:i + 1]
                        nc.vector.tensor_tensor(out=cv, in0=src, in1=gb1,
                                                op=Alu.add)
                        continue
                    else:
                        a = 256 // (2 * ci)
                        vo = old.rearrange("p (h a b c) -> p h a b c",
                                           h=NT, b=2, c=ci)
                        vn = new.rearrange("p (h a b c) -> p h a b c",
                                           h=NT, b=2, c=ci)
                        src = vo[:, :, :, 0, :]
                        tgt = vn[:, :, :, 1, :]
                        cv = cand.rearrange("p (h a c) -> p h a c",
                                            h=NT, c=ci)
                        gb = gb1.unsqueeze(3).broadcast_to([128, NT, a, ci])
                    if DPTT:
                        nc.vector.tensor_tensor(out=cv, in0=src, in1=gb, op=Alu.add)
                        nc.vector.tensor_tensor(out=tgt, in0=tgt, in1=cv, op=Alu.max)
                    else:
                        for h in range(NT):
                            nc.vector.scalar_tensor_tensor(
                                out=tgt[:, h], in0=src[:, h],
                                scalar=G32[:, h * N * N + col:h * N * N + col + 1],
                                in1=tgt[:, h], op0=Alu.add, op1=Alu.max)
            dmax = res_pool.tile([128, NT], f16, tag="dmax")
            nc.vector.tensor_reduce(
                out=dmax[:, :],
                in_=cand.rearrange("p (h s) -> p h s", h=NT)[:, :, 0:N],
                axis=mybir.AxisListType.X, op=Alu.max)
            nc.vector.scalar_tensor_tensor(
                out=loss_t[:, :],
                in0=dmax[:, :],
                scalar=-2.0,
                in1=s_all[:, :],
                op0=Alu.mult,
                op1=Alu.add,
            )
            nc.sync.dma_start(out=out_d[:, :], in_=loss_t[:, :])
    nc.compile()
    return nc


def kernel(y_true: np.ndarray, y_pred: np.ndarray) -> np.ndarray:
    from concourse.bass_utils import run_bass_kernel_spmd

    if "nc" not in _CACHE:
        _CACHE["nc"] = _build()
    nc = _CACHE["nc"]

    yt = np.ascontiguousarray(np.asarray(y_true, dtype=np.float32)).reshape(B, N * D)
    yp = np.ascontiguousarray(np.asarray(y_pred, dtype=np.float32)).reshape(B, N * D)

    in_maps = [
        {
            "yt": np.ascontiguousarray(yt[c * B_LOC:(c + 1) * B_LOC]),
            "yp": np.ascontiguousarray(yp[c * B_LOC:(c + 1) * B_LOC]),
        }
        for c in range(N_CORES)
    ]
    res = run_bass_kernel_spmd(nc, in_maps, list(range(N_CORES)), trace=TRACE)
    _CACHE["last_results"] = res
    vals = np.concatenate([np.asarray(r["out"], dtype=np.float64).reshape(-1)
                           for r in res.results])
    loss = vals.mean() / (D * N)
    return np.float32(loss)
